# revision 1
# baseline (speedup 1.0000x reference)
"""Trainium2 Bass kernel for nn_GATLayered (graph transformer w/ edge features).

Contract: kernel(**inputs) takes FULL numpy inputs (as produced by the
problem's setup_inputs), distributes across 8 NeuronCores, and returns the
FULL [B, N, D] float32 output.

Key algebraic restructure vs the reference:
  - E = edge_emb[edge_types] ([B,N,N,D], 256MB) is never materialized.
    Since edge_types has only ET=16 values, Ke = E @ Wk + bk collapses to a
    [16, D] table; the per-(i,j) edge score becomes a gather of
    qe[i,h,t] = Q_i . ke_table[t, head h], implemented as a one-hot matmul
    that accumulates straight into the attention-score PSUM.
  - adj masking is a rank-16 matmul (rowrep @ maskbias) accumulated into the
    same PSUM: scores never leave PSUM before the softmax exp.
  - Sharding: core c handles graph b=c//2; layer 0 is computed full-graph on
    both cores of a pair (redundant), layer 1 is query-sharded (128 rows per
    core). Cores never communicate.
"""

import os
import sys

import numpy as np

for _p in ("/opt/trn_rl_repo", "/root/.axon_site/_ro/trn_rl_repo"):
    if os.path.isdir(_p) and _p not in sys.path:
        sys.path.insert(0, _p)

import ml_dtypes

import concourse.bacc as bacc
import concourse.bass as bass
import concourse.mybir as mybir
import concourse.tile as tile
from concourse.bass_utils import run_bass_kernel_spmd

BF16 = mybir.dt.bfloat16
F32 = mybir.dt.float32
I32 = mybir.dt.int32
AF = mybir.ActivationFunctionType
OP = mybir.AluOpType

B, N, D, H, L, I = 4, 256, 256, 8, 2, 1024
V, ET, MAXPOS = 32000, 16, 512
DH = D // H
SCALE = 1.0 / float(np.sqrt(DH))
N_CORES = 8
KD = D // 128          # 2 contraction tiles over D
FT = I // 128          # 8 tiles over FFN dim

bf16 = ml_dtypes.bfloat16


def _ap(t, offset, dims):
    """Hand-built access pattern on a Tile tile or tensor handle."""
    h = t.tensor if hasattr(t, "tensor") else t
    return bass.AP(h, offset, dims)


def build_nc():
    nc = bacc.Bacc("TRN2", target_bir_lowering=False, debug=False,
                   num_devices=N_CORES)

    # ---------------- DRAM parameters (per-core) ----------------
    tok_d = nc.dram_tensor("tok", [V, D], F32, kind="ExternalInput")
    wid_d = nc.dram_tensor("wid", [128, 2], I32, kind="ExternalInput")
    pose_d = nc.dram_tensor("pose", [2, 128, D], F32, kind="ExternalInput")
    mbr_d = nc.dram_tensor("mbr", [16, 16 * 256], BF16, kind="ExternalInput")
    oh_d = nc.dram_tensor("oh", [128, (N // 8) * 256], BF16, kind="ExternalInput")
    wall_d = nc.dram_tensor("wall", [L, 128, 4 * KD * 256], BF16, kind="ExternalInput")
    w1all_d = nc.dram_tensor("w1all", [L, 128, KD * I], BF16, kind="ExternalInput")
    w2all_d = nc.dram_tensor("w2all", [L, 128, FT * 256], BF16, kind="ExternalInput")
    eeT_d = nc.dram_tensor("eeT", [D, ET], BF16, kind="ExternalInput")
    bq_d = nc.dram_tensor("bq", [128, L * KD], F32, kind="ExternalInput")
    bk_d = nc.dram_tensor("bk", [128, L * KD], F32, kind="ExternalInput")
    b1T_d = nc.dram_tensor("b1T", [128, L * FT], F32, kind="ExternalInput")
    boeff_d = nc.dram_tensor("boeff", [L, D], BF16, kind="ExternalInput")
    b2r_d = nc.dram_tensor("b2r", [L, D], BF16, kind="ExternalInput")
    lnp_d = nc.dram_tensor("lnp", [L * 4, D], F32, kind="ExternalInput")
    hm_d = nc.dram_tensor("hm", [128, KD * 8], BF16, kind="ExternalInput")
    bdm_d = nc.dram_tensor("bdm", [128, 128], BF16, kind="ExternalInput")
    rr_d = nc.dram_tensor("rr", [16, 128], BF16, kind="ExternalInput")
    idf_d = nc.dram_tensor("idf", [128, 128], F32, kind="ExternalInput")
    idb_d = nc.dram_tensor("idb", [128, 128], BF16, kind="ExternalInput")
    out_d = nc.dram_tensor("out", [128, D], F32, kind="ExternalOutput")

    with tile.TileContext(nc) as tc:
        with tc.tile_pool(name="pc", bufs=1) as pc, \
             tc.tile_pool(name="st", bufs=1) as st, \
             tc.tile_pool(name="pw", bufs=3) as pw, \
             tc.tile_pool(name="pps", bufs=2, space="PSUM") as pps, \
             tc.tile_pool(name="ppq", bufs=2, space="PSUM") as ppq, \
             tc.tile_pool(name="ppt", bufs=2, space="PSUM") as ppt, \
             tc.tile_pool(name="ppa", bufs=2, space="PSUM") as ppa:

            # ---------------- constants / weights to SBUF ----------------
            # DMA queue split: weights on the SP HWDGE queue (in layer order),
            # onehot/mask/LN constants on the ACT HWDGE queue, embedding
            # gathers on the Pool SWDGE queue.
            dma = nc.sync.dma_start
            dma2 = nc.scalar.dma_start

            # onehot first on its queue: the first attention group waits on it
            NSG = N // 8
            onehot = st.tile([128, NSG * 256], BF16)
            dma2(onehot[:], oh_d[:])
            mbr_sb = pc.tile([16, 16 * 256], BF16)
            dma2(mbr_sb[:], mbr_d[:])

            wall_sb = [pc.tile([128, 4 * KD * 256], BF16, tag=f"wall{l}", name=f"wall{l}") for l in range(L)]
            w1all_sb = [pc.tile([128, KD * I], BF16, tag=f"w1all{l}", name=f"w1all{l}") for l in range(L)]
            w2all_sb = [pc.tile([128, FT * 256], BF16, tag=f"w2all{l}", name=f"w2all{l}") for l in range(L)]
            for l in range(L):
                dma(wall_sb[l][:], wall_d[l])
                dma(w1all_sb[l][:], w1all_d[l])
                dma(w2all_sb[l][:], w2all_d[l])
            # slice views: cls 0..3 = wq, wk, wv, wo; each KD ktiles of 256
            wq_sb = [[wall_sb[l][:, (0 * KD + k) * 256:(0 * KD + k + 1) * 256] for k in range(KD)] for l in range(L)]
            wk_sb = [[wall_sb[l][:, (1 * KD + k) * 256:(1 * KD + k + 1) * 256] for k in range(KD)] for l in range(L)]
            wv_sb = [[wall_sb[l][:, (2 * KD + k) * 256:(2 * KD + k + 1) * 256] for k in range(KD)] for l in range(L)]
            wo_sb = [[wall_sb[l][:, (3 * KD + k) * 256:(3 * KD + k + 1) * 256] for k in range(KD)] for l in range(L)]
            w1_sb = [[w1all_sb[l][:, k * I:(k + 1) * I] for k in range(KD)] for l in range(L)]
            w2_sb = [[w2all_sb[l][:, f * 256:(f + 1) * 256] for f in range(FT)] for l in range(L)]

            eeT_sb = [pc.tile([128, ET], BF16, tag=f"eeT{k}", name=f"eeT{k}") for k in range(KD)]
            for k in range(KD):
                dma(eeT_sb[k][:], eeT_d[128 * k:128 * (k + 1), :])
            bq_sb = pc.tile([128, L * KD], F32)
            dma(bq_sb[:], bq_d[:])
            bk_sb = pc.tile([128, L * KD], F32)
            dma(bk_sb[:], bk_d[:])
            b1T_sb = pc.tile([128, L * FT], F32)
            dma(b1T_sb[:], b1T_d[:])
            boeff_sb = [pc.tile([1, D], BF16, tag=f"boe{l}", name=f"boe{l}") for l in range(L)]
            b2r_sb = [pc.tile([1, D], BF16, tag=f"b2r{l}", name=f"b2r{l}") for l in range(L)]
            for l in range(L):
                dma2(boeff_sb[l][:], boeff_d[l:l + 1, :])
                dma2(b2r_sb[l][:], b2r_d[l:l + 1, :])
            # LN params broadcast across partitions via 0-step DMA
            ln_sb = [[pc.tile([128, D], F32, tag=f"ln{l}{k}", name=f"ln{l}{k}") for k in range(4)] for l in range(L)]
            for l in range(L):
                for k in range(4):
                    dma2(ln_sb[l][k][:], _ap(lnp_d, (l * 4 + k) * D, [[0, 128], [1, D]]))
            hm_sb = pc.tile([128, KD * 8], BF16)
            dma2(hm_sb[:], hm_d[:])
            bdm_sb = pc.tile([128, 128], BF16)
            dma2(bdm_sb[:], bdm_d[:])
            rr_sb = pc.tile([16, 128], BF16)
            dma2(rr_sb[:], rr_d[:])
            idf_sb = pc.tile([128, 128], F32)
            dma2(idf_sb[:], idf_d[:])
            idb_sb = pc.tile([128, 128], BF16)
            dma2(idb_sb[:], idb_d[:])
            ones_sb = pc.tile([1, 128], BF16)
            nc.vector.memset(ones_sb[:], 1.0)
            eps_sb = pc.tile([128, 1], F32)
            nc.vector.memset(eps_sb[:], 1e-5)

            # ---------------- x0: embedding gather + positional ----------
            wid_sb = st.tile([128, 2], I32)
            dma(wid_sb[:], wid_d[:])
            x_nat = [st.tile([128, D], F32, tag=f"x{it}", name=f"x{it}") for it in range(2)]
            for it in range(2):
                xg = pw.tile([128, D], F32, tag="xg", name="xg")
                nc.gpsimd.indirect_dma_start(
                    out=xg[:], out_offset=None, in_=tok_d[:],
                    in_offset=bass.IndirectOffsetOnAxis(ap=wid_sb[:, it:it + 1], axis=0))
                pose_sb = pw.tile([128, D], F32, tag="pose", name="pose")
                dma(pose_sb[:], pose_d[it, :, :])
                nc.vector.tensor_tensor(out=x_nat[it][:], in0=xg[:], in1=pose_sb[:], op=OP.add)

            # ---------------- layers ----------------
            for l in range(L):
                n_q = N if l == 0 else 128
                n_it = n_q // 128      # query row-tiles
                NG = n_q // 16         # score groups
                lt = "a" if l == 0 else "b"

                # transpose x -> xT (bf16) [KD][128, N]
                xT = [st.tile([128, N], BF16, tag=f"xT{k}", name=f"xT{k}") for k in range(KD)]
                for k in range(KD):
                    for it in range(2):
                        tp = ppt.tile([128, 128], F32, tag="tp", name="tp")
                        nc.tensor.transpose(tp[:], x_nat[it][:, 128 * k:128 * (k + 1)], idf_sb[:])
                        nc.vector.tensor_copy(xT[k][:, 128 * it:128 * (it + 1)], tp[:])

                # K^T [nd][128, N] and V [jt][128, D], Qhm [nd][128, n_q*8]
                KT = [st.tile([128, N], BF16, tag=f"KT{k}", name=f"KT{k}") for k in range(KD)]
                for nt in range(KD):
                    ps = ppa.tile([128, N], F32, tag="acc", name="acc")
                    for k in range(KD):
                        nc.tensor.matmul(ps[:], wk_sb[l][k][:, 128 * nt:128 * (nt + 1)],
                                         xT[k][:], start=(k == 0), stop=(k == KD - 1))
                    nc.vector.tensor_scalar(out=KT[nt][:], in0=ps[:],
                                            scalar1=bk_sb[:, l * KD + nt:l * KD + nt + 1],
                                            scalar2=None, op0=OP.add)
                Vn = [st.tile([128, D], BF16, tag=f"V{j}", name=f"V{j}") for j in range(2)]
                for jt in range(2):
                    ps = ppa.tile([128, D], F32, tag="acc", name="acc")
                    for k in range(KD):
                        nc.tensor.matmul(ps[:], xT[k][:, 128 * jt:128 * (jt + 1)],
                                         wv_sb[l][k][:], start=(k == 0), stop=(k == KD - 1))
                    nc.vector.tensor_copy(Vn[jt][:], ps[:])
                Qhm = [st.tile([128, n_q * 8], BF16, tag=f"Qhm{k}", name=f"Qhm{k}") for k in range(KD)]
                for nt in range(KD):
                    ps = ppa.tile([128, n_q], F32, tag="acc", name="acc")
                    for k in range(KD):
                        nc.tensor.matmul(ps[:], wq_sb[l][k][:, 128 * nt:128 * (nt + 1)],
                                         xT[k][:, :n_q], start=(k == 0), stop=(k == KD - 1))
                    pstep = ps[:].ap[0][0]
                    in0 = _ap(ps, 0, [[pstep, 128], [1, n_q], [0, 8]])
                    in1 = _ap(hm_sb, nt * 8, [[hm_sb[:].ap[0][0], 128], [0, n_q], [1, 8]])
                    outap = _ap(Qhm[nt], 0, [[Qhm[nt][:].ap[0][0], 128], [8, n_q], [1, 8]])
                    nc.vector.scalar_tensor_tensor(
                        out=outap, in0=in0, scalar=bq_sb[:, l * KD + nt:l * KD + nt + 1],
                        in1=in1, op0=OP.add, op1=OP.mult)

                # ke table -> replicated ke_rep [nd][128, 128]
                ke_rep = [st.tile([128, 128], BF16, tag=f"ker{k}", name=f"ker{k}") for k in range(KD)]
                for nt in range(KD):
                    ps = ppq.tile([128, ET], F32, tag="qe", name="qe")
                    for k in range(KD):
                        nc.tensor.matmul(ps[:], wk_sb[l][k][:, 128 * nt:128 * (nt + 1)],
                                         eeT_sb[k][:], start=(k == 0), stop=(k == KD - 1))
                    keT = pw.tile([128, ET], BF16, tag="keT", name="keT")
                    nc.vector.tensor_scalar(out=keT[:], in0=ps[:],
                                            scalar1=bk_sb[:, l * KD + nt:l * KD + nt + 1],
                                            scalar2=None, op0=OP.add)
                    kp = keT[:].ap[0][0]
                    in_ = _ap(keT, 0, [[kp, 128], [0, 8], [1, ET]])
                    outap = _ap(ke_rep[nt], 0, [[ke_rep[nt][:].ap[0][0], 128], [ET, 8], [1, ET]])
                    nc.vector.tensor_copy(outap, in_)

                # ---------- attention groups ----------
                PT_all = [st.tile([128, NG * 128], BF16, tag=f"PT{j}", name=f"PT{j}") for j in range(2)]
                for g in range(NG):
                    # qe for the 2 subgroups of this group -> bd-masked lhsT
                    qe_ps = ppq.tile([128, 128], F32, tag="qe", name="qe")
                    for s2 in range(2):
                        sg = 2 * g + s2
                        for k in range(KD):
                            nc.tensor.matmul(qe_ps[:, 64 * s2:64 * (s2 + 1)],
                                             ke_rep[k][:], Qhm[k][:, 64 * sg:64 * (sg + 1)],
                                             start=(k == 0), stop=(k == KD - 1))
                    bd_sb = pw.tile([128, 128], BF16, tag="bd", name="bd")
                    nc.vector.tensor_tensor(out=bd_sb[:], in0=qe_ps[:], in1=bdm_sb[:], op=OP.mult)

                    s_ps = pps.tile([128, 256], F32, tag="s", name="s")
                    for k in range(KD):
                        nc.tensor.matmul(s_ps[:], Qhm[k][:, 128 * g:128 * (g + 1)], KT[k][:],
                                         start=(k == 0), stop=False)
                    for s2 in range(2):
                        sg = 2 * g + s2
                        nc.tensor.matmul(s_ps[64 * s2:64 * (s2 + 1), :],
                                         bd_sb[:, 64 * s2:64 * (s2 + 1)],
                                         onehot[:, 256 * sg:256 * (sg + 1)],
                                         start=False, stop=False,
                                         tile_position=(0, 64 * s2))
                    nc.tensor.matmul(s_ps[:], rr_sb[:], mbr_sb[:, 256 * g:256 * (g + 1)],
                                     start=False, stop=True)

                    Pn = pw.tile([128, 256], BF16, tag="Pn", name="Pn")
                    rsum = pw.tile([128, 1], F32, tag="rsum", name="rsum")
                    nc.scalar.activation(Pn[:], s_ps[:], AF.Exp, scale=SCALE,
                                         accum_out=rsum[:])
                    rrec = pw.tile([128, 1], F32, tag="rrec", name="rrec")
                    nc.vector.reciprocal(rrec[:], rsum[:])
                    nc.vector.tensor_scalar(out=Pn[:], in0=Pn[:], scalar1=rrec[:, 0:1],
                                            scalar2=None, op0=OP.mult)
                    for jt in range(2):
                        tp = ppt.tile([128, 128], BF16, tag="tp", name="tp")
                        nc.tensor.transpose(tp[:], Pn[:, 128 * jt:128 * (jt + 1)], idb_sb[:])
                        nc.vector.tensor_copy(PT_all[jt][:, 128 * g:128 * (g + 1)], tp[:])

                # ---------- context ----------
                ctxT = [st.tile([128, n_q], BF16, tag=f"ctxT{d}", name=f"ctxT{d}") for d in range(2)]
                for dt in range(2):
                    cps = ppa.tile([128, n_q], F32, tag="acc", name="acc")
                    for h4 in range(4):
                        h = dt * 4 + h4
                        for jt in range(2):
                            rhs = _ap(PT_all[jt], h,
                                      [[PT_all[jt][:].ap[0][0], 128], [128, NG], [8, 16]])
                            nc.tensor.matmul(cps[32 * h4:32 * (h4 + 1), :],
                                             Vn[jt][:, DH * h:DH * (h + 1)], rhs,
                                             start=(jt == 0), stop=(jt == 1),
                                             tile_position=(0, 32 * h4))
                    nc.vector.tensor_copy(ctxT[dt][:], cps[:])

                # ---------- out-projection + residual + LN1 ----------
                x1 = [st.tile([128, D], F32, tag=f"x1{it}", name=f"x1{it}") for it in range(n_it)]
                for it in range(n_it):
                    ps = ppa.tile([128, D], F32, tag="acc", name="acc")
                    for dt in range(2):
                        nc.tensor.matmul(ps[:], ctxT[dt][:, 128 * it:128 * (it + 1)],
                                         wo_sb[l][dt][:], start=(dt == 0), stop=False)
                    nc.tensor.matmul(ps[:], ones_sb[:], boeff_sb[l][:],
                                     start=False, stop=True)
                    xatt = pw.tile([128, D], F32, tag="xatt", name="xatt")
                    nc.vector.tensor_tensor(out=xatt[:], in0=ps[:], in1=x_nat[it][:], op=OP.add)
                    _layernorm(nc, pw, xatt, x1[it], ln_sb[l][0], ln_sb[l][1], eps_sb)

                # ---------- FFN ----------
                x1T = [st.tile([128, n_q], BF16, tag=f"x1T{k}", name=f"x1T{k}") for k in range(KD)]
                for k in range(KD):
                    for it in range(n_it):
                        tp = ppt.tile([128, 128], F32, tag="tp", name="tp")
                        nc.tensor.transpose(tp[:], x1[it][:, 128 * k:128 * (k + 1)], idf_sb[:])
                        nc.vector.tensor_copy(x1T[k][:, 128 * it:128 * (it + 1)], tp[:])
                hT = [st.tile([128, n_q], BF16, tag=f"hT{f}", name=f"hT{f}") for f in range(FT)]
                for ft in range(FT):
                    ps = ppa.tile([128, n_q], F32, tag="acc", name="acc")
                    for k in range(KD):
                        nc.tensor.matmul(ps[:], w1_sb[l][k][:, 128 * ft:128 * (ft + 1)],
                                         x1T[k][:], start=(k == 0), stop=(k == KD - 1))
                    nc.vector.tensor_scalar(out=hT[ft][:], in0=ps[:],
                                            scalar1=b1T_sb[:, l * FT + ft:l * FT + ft + 1],
                                            scalar2=0.0, op0=OP.add, op1=OP.max)
                xo = [st.tile([128, D], F32, tag=f"xo{it}", name=f"xo{it}") for it in range(n_it)]
                for it in range(n_it):
                    ps = ppa.tile([128, D], F32, tag="acc", name="acc")
                    for ft in range(FT):
                        nc.tensor.matmul(ps[:], hT[ft][:, 128 * it:128 * (it + 1)],
                                         w2_sb[l][ft][:], start=(ft == 0), stop=False)
                    nc.tensor.matmul(ps[:], ones_sb[:], b2r_sb[l][:],
                                     start=False, stop=True)
                    x2pre = pw.tile([128, D], F32, tag="x2pre", name="x2pre")
                    nc.vector.tensor_tensor(out=x2pre[:], in0=ps[:], in1=x1[it][:], op=OP.add)
                    _layernorm(nc, pw, x2pre, xo[it], ln_sb[l][2], ln_sb[l][3], eps_sb)
                x_nat = xo

            dma(out_d[:], x_nat[0][:])

    nc.compile()
    return nc


def _layernorm(nc, pw, xin, xout, lns_bc, lnb_bc, eps_sb):
    """LN over the free dim: xout = (xin - mean)/sqrt(var+eps) * lns + lnb.

    rstd = exp(-0.5*ln(var+eps)) keeps the scalar engine on the single
    natural_log_exp_and_others activation table (no table reloads)."""
    st6 = pw.tile([128, 6], F32, tag="st6", name="st6")
    nc.vector.bn_stats(st6[:], xin[:])
    st2 = pw.tile([128, 2], F32, tag="st2", name="st2")
    nc.vector.bn_aggr(st2[:], st6[:])
    lnv = pw.tile([128, 1], F32, tag="lnv", name="lnv")
    nc.scalar.activation(lnv[:], st2[:, 1:2], AF.Ln, bias=eps_sb[:, 0:1])
    rstd = pw.tile([128, 1], F32, tag="rstd", name="rstd")
    nc.scalar.activation(rstd[:], lnv[:], AF.Exp, scale=-0.5)
    u = pw.tile([128, D], F32, tag="lnu", name="lnu")
    nc.vector.scalar_tensor_tensor(out=u[:], in0=xin[:], scalar=st2[:, 0:1],
                                   in1=lns_bc[:], op0=OP.subtract, op1=OP.mult)
    nc.vector.scalar_tensor_tensor(out=xout[:], in0=u[:], scalar=rstd[:, 0:1],
                                   in1=lnb_bc[:], op0=OP.mult, op1=OP.add)


def prep_inputs(inputs):
    """Host-side sharding/layout prep. Returns per-core input dicts."""
    f32 = np.float32
    tok_emb = np.ascontiguousarray(inputs["tok_emb"], f32)
    pos_emb = np.asarray(inputs["pos_emb"], f32)
    edge_emb = np.asarray(inputs["edge_emb"], f32)
    word_ids = np.asarray(inputs["word_ids"])
    adj = np.asarray(inputs["adj"])
    edge_types = np.asarray(inputs["edge_types"])

    Wq = np.asarray(inputs["Wq"], f32)
    Wk = np.asarray(inputs["Wk"], f32)
    Wv = np.asarray(inputs["Wv"], f32)
    Wo = np.asarray(inputs["Wo"], f32)
    W1 = np.asarray(inputs["W1"], f32)
    W2 = np.asarray(inputs["W2"], f32)
    bqv = np.asarray(inputs["bq"], f32)
    bkv = np.asarray(inputs["bk"], f32)
    bvv = np.asarray(inputs["bv"], f32)
    bov = np.asarray(inputs["bo"], f32)
    b1v = np.asarray(inputs["b1"], f32)
    b2v = np.asarray(inputs["b2"], f32)

    # shared (core-independent) tensors
    shared = {}
    shared["tok"] = tok_emb
    # wall[l][p, (cls*KD+k)*256 + j] = W_cls[l, 128k+p, j], cls 0..3 = q,k,v,o
    wall = np.stack([
        np.concatenate([W.reshape(L, KD, 128, D) for W in (Wq, Wk, Wv, Wo)],
                       axis=1)                      # [L, 4*KD, 128, D]
        .transpose(0, 2, 1, 3).reshape(L, 128, 4 * KD * D)[ll]
        for ll in range(L)])
    shared["wall"] = np.ascontiguousarray(wall).astype(bf16)
    shared["w1all"] = np.ascontiguousarray(
        W1.reshape(L, KD, 128, I).transpose(0, 2, 1, 3).reshape(L, 128, KD * I)).astype(bf16)
    shared["w2all"] = np.ascontiguousarray(
        W2.reshape(L, FT, 128, D).transpose(0, 2, 1, 3).reshape(L, 128, FT * D)).astype(bf16)
    shared["eeT"] = np.ascontiguousarray(edge_emb.T).astype(bf16)
    shared["bq"] = np.ascontiguousarray(bqv.reshape(L, KD, 128).transpose(2, 0, 1).reshape(128, L * KD))
    shared["bk"] = np.ascontiguousarray(bkv.reshape(L, KD, 128).transpose(2, 0, 1).reshape(128, L * KD))
    shared["b1T"] = np.ascontiguousarray(b1v.reshape(L, FT, 128).transpose(2, 0, 1).reshape(128, L * FT))
    boeff = bov + np.einsum("ld,lde->le", bvv, Wo)  # [L, D]
    shared["boeff"] = boeff.astype(bf16)
    shared["b2r"] = b2v.astype(bf16)
    lnp = np.stack([np.asarray(inputs["ln1_s"], f32), np.asarray(inputs["ln1_b"], f32),
                    np.asarray(inputs["ln2_s"], f32), np.asarray(inputs["ln2_b"], f32)], axis=1)
    shared["lnp"] = np.ascontiguousarray(lnp.reshape(L * 4, D))
    hm = np.zeros((128, KD * 8), f32)
    for nt in range(KD):
        for p in range(128):
            hm[p, nt * 8 + (nt * 128 + p) // DH % H] = 1.0
    # head of global n = n // DH; for nt tile: (nt*128+p)//DH
    hm[:] = 0.0
    for nt in range(KD):
        for p in range(128):
            hm[p, nt * 8 + ((nt * 128 + p) // DH)] = 1.0
    shared["hm"] = hm.astype(bf16)
    bdm = np.zeros((128, 128), f32)
    for p in range(128):
        for c in range(128):
            if p // 16 == (c % 64) // 8:
                bdm[p, c] = 1.0
    shared["bdm"] = bdm.astype(bf16)
    rr = np.zeros((16, 128), f32)
    for r in range(16):
        rr[r, r * 8:(r + 1) * 8] = 1.0
    shared["rr"] = rr.astype(bf16)
    shared["idf"] = np.eye(128, dtype=f32)
    shared["idb"] = np.eye(128, dtype=f32).astype(bf16)

    in_maps = []
    for core in range(N_CORES):
        b, half = core // 2, core % 2
        own = np.arange(half * 128, half * 128 + 128)
        other = np.arange((1 - half) * 128, (1 - half) * 128 + 128)
        perm = np.concatenate([own, other])
        m = dict(shared)
        m["wid"] = np.ascontiguousarray(
            word_ids[b][perm].reshape(2, 128).T.astype(np.int32))
        m["pose"] = np.ascontiguousarray(pos_emb[:N][perm].reshape(2, 128, D))
        adj_l = adj[b][np.ix_(perm, perm)]
        mb = np.where(adj_l > 0, 0.0, -30000.0).astype(f32)
        # mbr[p, g*256+j] = mb[16g+p, j]
        mbr = np.ascontiguousarray(mb.reshape(16, 16, 256).transpose(1, 0, 2).reshape(16, 16 * 256))
        m["mbr"] = mbr.astype(bf16)
        etp = edge_types[b][np.ix_(perm, perm)]          # [256, 256]
        # oh[t+16r, sg*256+j] = (etp[8sg+r, j] == t)
        m4 = (etp.reshape(N // 8, 8, N)[None, :, :, :]
              == np.arange(ET)[:, None, None, None])     # [t, sg, r, j]
        m["oh"] = np.ascontiguousarray(
            m4.transpose(2, 0, 1, 3).reshape(128, (N // 8) * N)).astype(bf16)
        in_maps.append(m)
    return in_maps


_NC_CACHE = {}


def get_nc():
    if "nc" not in _NC_CACHE:
        _NC_CACHE["nc"] = build_nc()
    return _NC_CACHE["nc"]


def kernel(**inputs):
    nc = get_nc()
    in_maps = prep_inputs(inputs)
    res = run_bass_kernel_spmd(nc, in_maps, list(range(N_CORES)))
    out = np.zeros((B, N, D), np.float32)
    for core in range(N_CORES):
        b, half = core // 2, core % 2
        out[b, half * 128:half * 128 + 128] = res.results[core]["out"]
    return out

